# revision 8
# baseline (speedup 1.0000x reference)
"""MoE routing kernel (nn_MoE_52037823758984) for 8x Trainium2 NeuronCores.

Computes out[i] = expert_{route[i]}(x[i]) where each expert is a Linear(10,10):
    y0 = x @ W1.T + b1 ; y1 = x @ W2.T + b2 ; out = where(route==0, y0, y1)

Sharding: data-parallel over the token dim; each of the 8 cores processes
N/8 = 262144 tokens.

Shipped algorithm (build_moe_pe): TensorEngine block-diagonal matmul on a
host-pre-planarized bf16 layout.
  By linearity  out = x@W1.T + (r*x)@Wd.T + r*bd + b1  (Wd=W2-W1, bd=b2-b1).
  The host builds, per token, 21 augmented rows [x(10), r*x(10), r(1)] in
  bf16, laid out feature-planar as 6 independent token streams of 21 rows
  each -> a [126, F] SBUF tile per chunk. One K=126, M=60 block-diagonal
  matmul per 512 columns then computes all 6 streams' outputs at once
  (~215ns per 3072 tokens on the PE), with b1 fused into the PSUM->SBUF
  drain (per-partition bias on the Scalar engine / tensor_scalar_add on the
  Vector engine, alternating). Outputs leave planar bf16; the host
  un-planarizes and casts to f32. Device-side traffic is ~16.3 MB/core of
  contiguous >=0.5MB DMAs, making the kernel DMA-bound (~50us/core) instead
  of DVE-bound (the previous all-DVE variant measured ~470-1000us).

bf16 is safe here: tolerance is 2e-2 max-rel; bf16 input+output rounding
contributes ~5e-3.

The previous DVE implementation (build_moe_v3) is kept for A/B reference.
"""

import math

import numpy as np
import ml_dtypes

import concourse.bacc as bacc
import concourse.mybir as mybir
from concourse.tile import TileContext
from concourse.bass_utils import run_bass_kernel_spmd

F32 = mybir.dt.float32
BF16 = mybir.dt.bfloat16
I32 = mybir.dt.int32
ALU = mybir.AluOpType
NPBF16 = ml_dtypes.bfloat16

N_CORES = 8
P = 128

# PE-kernel geometry (aug variant)
D = 10          # feature dim
AUG = 21        # augmented rows per token: x(10), r*x(10), r(1)
S = 6           # token streams per PE column (6*21 = 126 <= 128)
KDIM = S * AUG  # 126 contraction rows
M = S * D       # 60 output rows
FTILE = 4096    # columns per SBUF tile (1 MB input DMAs)
NCOL = 512      # columns per matmul (one PSUM bank)

# sorted variant geometry: tokens sorted by route on host, so streams are
# expert-pure and only the 10 x rows ship per token.
S2 = 12          # streams (12*10 = 120 partitions)
K2 = S2 * D      # 120
F2 = 2048        # columns per tile
NT2 = 11         # tiles: 132 streams >= worst-case ceil(n0/F2)+ceil(n1/F2)

# v2 geometry: mixed tile widths (10x2048 + 1x1536 cols) so padded capacity
# 264192 just covers 262144 tokens + max 2047 boundary pad; paired 1MB DMAs;
# single upfront bias load; balanced 2-op drains per 2048-col PSUM block.
TILE_F = [F2] * 10 + [1536]          # logical tile widths (cols)
NPAIR = 5                            # tiles 0..9 pair into 1MB DMAs
FT = TILE_F[-1]                      # tail width
ACT_COLS = 1152                      # ACT's share of a 2048-col drain
ACT_COLS_T = 864                     # ACT's share of the 1536-col tail


def build_moe_pe(nt, reps=1):
    """Per-core program: nt tiles of [KDIM, FTILE] bf16 -> [M, FTILE] bf16."""
    AF = mybir.ActivationFunctionType
    nc = bacc.Bacc("TRN2", target_bir_lowering=False, debug=False,
                   num_devices=N_CORES)
    xa = nc.dram_tensor("xa", [nt, KDIM, FTILE], BF16, kind="ExternalInput")
    wm = nc.dram_tensor("wm", [KDIM, M], BF16, kind="ExternalInput")
    bv = nc.dram_tensor("bv", [M, 1], F32, kind="ExternalInput")
    ov = nc.dram_tensor("out", [nt, M, FTILE], BF16, kind="ExternalOutput")

    nmm = FTILE // NCOL
    with TileContext(nc) as tc:
        with tc.tile_pool(name="const", bufs=1) as cpool, \
             tc.tile_pool(name="sbuf", bufs=3) as pool, \
             tc.tile_pool(name="psum", bufs=8, space="PSUM") as ppool:
            wt = cpool.tile([KDIM, M], BF16)
            bt = cpool.tile([M, 1], F32)
            nc.sync.dma_start(out=wt[:], in_=wm[:])
            nc.sync.dma_start(out=bt[:], in_=bv[:])
            for _ in range(reps):
                for i in range(nt):
                    xt = pool.tile([KDIM, FTILE], BF16, tag="xt")
                    nc.sync.dma_start(out=xt[:], in_=xa[i])
                    ot = pool.tile([M, FTILE], BF16, tag="ot")
                    for m in range(nmm):
                        ps = ppool.tile([M, NCOL], F32, tag="ps")
                        nc.tensor.matmul(out=ps[:], lhsT=wt[:],
                                         rhs=xt[:, m * NCOL:(m + 1) * NCOL],
                                         start=True, stop=True)
                        sl = ot[:, m * NCOL:(m + 1) * NCOL]
                        # drain PSUM + add b1 + cast to bf16, alternating
                        # engines so neither becomes the bottleneck
                        if m % 2 == 0:
                            nc.scalar.activation(out=sl, in_=ps[:],
                                                 func=AF.Identity,
                                                 bias=bt[:], scale=1.0)
                        else:
                            nc.vector.tensor_scalar_add(out=sl, in0=ps[:],
                                                        scalar1=bt[:])
                    nc.sync.dma_start(out=ov[i], in_=ot[:])
    nc.compile()
    return nc


def build_moe_sorted(reps=1):
    """Per-core program, sorted variant: NT2 tiles of [K2, F2] bf16 x, with
    per-tile stationary weights [K2, K2] and per-tile bias [K2, 1] (the
    stream->expert assignment is data, so the program is route-independent).
    """
    AF = mybir.ActivationFunctionType
    nc = bacc.Bacc("TRN2", target_bir_lowering=False, debug=False,
                   num_devices=N_CORES)
    xa = nc.dram_tensor("xa", [NT2, K2, F2], BF16, kind="ExternalInput")
    wm = nc.dram_tensor("wm", [NT2, K2, K2], BF16, kind="ExternalInput")
    bv = nc.dram_tensor("bv", [NT2, K2, 1], F32, kind="ExternalInput")
    ov = nc.dram_tensor("out", [NT2, K2, F2], BF16, kind="ExternalOutput")

    nmm = F2 // NCOL
    with TileContext(nc) as tc:
        with tc.tile_pool(name="sbuf", bufs=3) as pool, \
             tc.tile_pool(name="psum", bufs=8, space="PSUM") as ppool:
            for _ in range(reps):
                for i in range(NT2):
                    wt = pool.tile([K2, K2], BF16, tag="wt")
                    bt = pool.tile([K2, 1], F32, tag="bt")
                    xt = pool.tile([K2, F2], BF16, tag="xt")
                    nc.sync.dma_start(out=wt[:], in_=wm[i])
                    nc.sync.dma_start(out=bt[:], in_=bv[i])
                    nc.sync.dma_start(out=xt[:], in_=xa[i])
                    ot = pool.tile([K2, F2], BF16, tag="ot")
                    for m in range(nmm):
                        ps = ppool.tile([K2, NCOL], F32, tag="ps")
                        nc.tensor.matmul(out=ps[:], lhsT=wt[:],
                                         rhs=xt[:, m * NCOL:(m + 1) * NCOL],
                                         start=True, stop=True)
                        sl = ot[:, m * NCOL:(m + 1) * NCOL]
                        if m % 2 == 0:
                            nc.scalar.activation(out=sl, in_=ps[:],
                                                 func=AF.Identity,
                                                 bias=bt[:], scale=1.0)
                        else:
                            nc.vector.tensor_scalar_add(out=sl, in0=ps[:],
                                                        scalar1=bt[:])
                    nc.sync.dma_start(out=ov[i], in_=ot[:])
    nc.compile()
    return nc


def build_moe_sorted2(reps=1):
    """v2: mixed tile widths, paired DMAs, upfront bias, balanced drains."""
    AF = mybir.ActivationFunctionType
    nc = bacc.Bacc("TRN2", target_bir_lowering=False, debug=False,
                   num_devices=N_CORES)
    xm = nc.dram_tensor("xm", [NPAIR, K2, 2 * F2], BF16,
                        kind="ExternalInput")
    xt_d = nc.dram_tensor("xt", [K2, FT], BF16, kind="ExternalInput")
    wmm = nc.dram_tensor("wmm", [NPAIR, K2, 2 * K2], BF16,
                         kind="ExternalInput")
    wmt = nc.dram_tensor("wmt", [K2, K2], BF16, kind="ExternalInput")
    bva_d = nc.dram_tensor("bva", [K2, len(TILE_F)], F32,
                           kind="ExternalInput")
    om = nc.dram_tensor("om", [NPAIR, K2, 2 * F2], BF16,
                        kind="ExternalOutput")
    ot_d = nc.dram_tensor("ot", [K2, FT], BF16, kind="ExternalOutput")

    with TileContext(nc) as tc:
        with tc.tile_pool(name="const", bufs=1) as cpool, \
             tc.tile_pool(name="sbuf", bufs=3) as pool, \
             tc.tile_pool(name="psum", bufs=2, space="PSUM") as ppool:
            bva = cpool.tile([K2, len(TILE_F)], F32)
            nc.sync.dma_start(out=bva[:], in_=bva_d[:])

            def subtile(wt_ap, xt_ap, ot_ap, tile_idx, cols, act_cols):
                ps = ppool.tile([K2, F2], F32, tag="ps")
                for m in range(cols // NCOL):
                    nc.tensor.matmul(out=ps[:, m * NCOL:(m + 1) * NCOL],
                                     lhsT=wt_ap,
                                     rhs=xt_ap[:, m * NCOL:(m + 1) * NCOL],
                                     start=True, stop=True)
                bb = bva[:, tile_idx:tile_idx + 1]
                nc.scalar.activation(out=ot_ap[:, :act_cols],
                                     in_=ps[:, :act_cols],
                                     func=AF.Identity, bias=bb, scale=1.0)
                nc.vector.tensor_scalar_add(out=ot_ap[:, act_cols:cols],
                                            in0=ps[:, act_cols:cols],
                                            scalar1=bb)

            for _ in range(reps):
                for p in range(NPAIR):
                    wt2 = pool.tile([K2, 2 * K2], BF16, tag="wt")
                    xt = pool.tile([K2, 2 * F2], BF16, tag="xt")
                    nc.sync.dma_start(out=wt2[:], in_=wmm[p])
                    nc.sync.dma_start(out=xt[:], in_=xm[p])
                    ot = pool.tile([K2, 2 * F2], BF16, tag="ot")
                    for h in range(2):
                        subtile(wt2[:, h * K2:(h + 1) * K2],
                                xt[:, h * F2:(h + 1) * F2],
                                ot[:, h * F2:(h + 1) * F2],
                                2 * p + h, F2, ACT_COLS)
                    nc.sync.dma_start(out=om[p], in_=ot[:])
                # tail tile (FT cols)
                wtl = pool.tile([K2, K2], BF16, tag="wtl")
                xtl = pool.tile([K2, FT], BF16, tag="xtl")
                nc.sync.dma_start(out=wtl[:], in_=wmt[:])
                nc.sync.dma_start(out=xtl[:], in_=xt_d[:])
                otl = pool.tile([K2, FT], BF16, tag="otl")
                subtile(wtl[:], xtl[:], otl[:], len(TILE_F) - 1, FT,
                        ACT_COLS_T)
                nc.sync.dma_start(out=ot_d[:], in_=otl[:])
    nc.compile()
    return nc


def _slot_table(n0, tc_tokens):
    """Slot layout for v2: returns per-slot (src_start, length, expert).

    Slots are the 12*len(TILE_F) streams in order; group0 fills slots first
    (last one partially), then group1 starts at a fresh slot.
    """
    caps = []
    for f in TILE_F:
        caps += [f] * S2
    slots = []
    src = 0
    n1 = tc_tokens - n0
    for cap in caps:
        if src < n0:
            ln = min(cap, n0 - src)
            slots.append((src, ln, 0))
            src += ln
        elif src < tc_tokens:
            ln = min(cap, tc_tokens - src)
            slots.append((src, ln, 1))
            src += ln
        else:
            slots.append((src, 0, 0))
    assert src == tc_tokens, (src, n0, n1)
    return slots


def make_sorted2_inputs(x, route, tc_tokens, W1, b1, W2, b2):
    """Host prep for v2. Returns (in_maps, perm_infos)."""
    WT = np.stack([W1.T.astype(NPBF16), W2.T.astype(NPBF16)])
    BB = np.stack([b1, b2]).astype(np.float32)
    ntile = len(TILE_F)
    in_maps, perms = [], []
    for c in range(N_CORES):
        sl = slice(c * tc_tokens, (c + 1) * tc_tokens)
        xs, rs = x[sl], route[sl]
        perm = np.argsort(rs, kind="stable")
        n0 = int(np.count_nonzero(rs == 0))
        xsorted = np.ascontiguousarray(xs[perm]).astype(NPBF16)
        slots = _slot_table(n0, tc_tokens)

        # planar buffers per tile, then split into paired/tail dram arrays
        tiles = []
        wm_tiles = np.zeros((ntile, K2, K2), NPBF16)
        bva = np.empty((K2, ntile), np.float32)
        si = 0
        for t, f in enumerate(TILE_F):
            buf = np.zeros((S2, f, D), NPBF16)
            for s in range(S2):
                src, ln, e = slots[si]
                if ln:
                    buf[s, :ln] = xsorted[src:src + ln]
                wm_tiles[t, s * D:(s + 1) * D, s * D:(s + 1) * D] = WT[e]
                bva[s * D:(s + 1) * D, t] = BB[e]
                si += 1
            # [S2, f, D] -> [S2, D, f] -> [K2, f]
            tiles.append(np.ascontiguousarray(
                buf.transpose(0, 2, 1)).reshape(K2, f))
        xm = np.stack([np.concatenate([tiles[2 * p], tiles[2 * p + 1]], 1)
                       for p in range(NPAIR)])
        wmm = np.stack(
            [np.concatenate([wm_tiles[2 * p], wm_tiles[2 * p + 1]], 1)
             for p in range(NPAIR)])
        in_maps.append({"xm": xm, "xt": tiles[-1], "wmm": wmm,
                        "wmt": wm_tiles[-1], "bva": bva})
        perms.append((perm, n0))
    return in_maps, perms


def unsort2_output(res_c, perm_info, tc_tokens):
    """v2 device outputs -> [tc_tokens, D] f32 in original order."""
    perm, n0 = perm_info
    om = np.asarray(res_c["om"])
    otl = np.asarray(res_c["ot"])
    flats = []
    for t, f in enumerate(TILE_F):
        if t < 2 * NPAIR:
            planar = om[t // 2][:, (t % 2) * F2:(t % 2) * F2 + f]
        else:
            planar = otl
        # [K2, f] -> [S2, D, f] -> [S2, f, D]
        flats.append(planar.reshape(S2, D, f).transpose(0, 2, 1))
    slots = _slot_table(n0, tc_tokens)
    ysorted = np.empty((tc_tokens, D), np.float32)
    si = 0
    for t, f in enumerate(TILE_F):
        for s in range(S2):
            src, ln, _ = slots[si]
            if ln:
                ysorted[src:src + ln] = flats[t][s, :ln]
            si += 1
    y = np.empty_like(ysorted)
    y[perm] = ysorted
    return y


def make_sorted_inputs(x, route, tc_tokens, W1, b1, W2, b2):
    """Host: per-core route-sort + planarize; returns (in_maps, perms, n0s)."""
    WT = np.stack([W1.T.astype(NPBF16), W2.T.astype(NPBF16)])  # [2, D, D]
    BB = np.stack([b1, b2]).astype(np.float32)                 # [2, D]
    n_streams = NT2 * S2
    in_maps, perms = [], []
    for c in range(N_CORES):
        sl = slice(c * tc_tokens, (c + 1) * tc_tokens)
        xs, rs = x[sl], route[sl]
        perm = np.argsort(rs, kind="stable")
        n0 = int(np.count_nonzero(rs == 0))
        n1 = tc_tokens - n0
        g0 = -(-n0 // F2)
        g1 = -(-n1 // F2)
        assert g0 + g1 <= n_streams
        xsorted = xs[perm]
        buf = np.zeros((n_streams * F2, D), NPBF16)
        buf[:n0] = xsorted[:n0]
        buf[g0 * F2:g0 * F2 + n1] = xsorted[n0:]
        xa = np.ascontiguousarray(
            buf.reshape(NT2, S2, F2, D).transpose(0, 1, 3, 2)
        ).reshape(NT2, K2, F2)
        ex = np.zeros(n_streams, np.int64)
        ex[g0:g0 + g1] = 1
        wm = np.zeros((NT2, K2, K2), NPBF16)
        bv = np.empty((NT2, K2, 1), np.float32)
        for i in range(NT2):
            for s in range(S2):
                e = ex[i * S2 + s]
                wm[i, s * D:(s + 1) * D, s * D:(s + 1) * D] = WT[e]
                bv[i, s * D:(s + 1) * D, 0] = BB[e]
        in_maps.append({"xa": xa, "wm": wm, "bv": bv})
        perms.append((perm, n0, g0))
    return in_maps, perms


def unsort_output(o, perm_info, tc_tokens):
    """[NT2, K2, F2] planar bf16 -> [tc_tokens, D] f32 in original order."""
    perm, n0, g0 = perm_info
    flat = np.asarray(o).reshape(NT2, S2, D, F2).transpose(0, 1, 3, 2)
    flat = flat.reshape(NT2 * S2 * F2, D)
    ysorted = np.empty((tc_tokens, D), np.float32)
    ysorted[:n0] = flat[:n0]
    ysorted[n0:] = flat[g0 * F2:g0 * F2 + (tc_tokens - n0)]
    y = np.empty_like(ysorted)
    y[perm] = ysorted
    return y


def make_pe_weights(W1, b1, W2, b2):
    """Block-diagonal stationary [KDIM, M] bf16 and bias [M, 1] f32."""
    Wd = W2 - W1
    bd = b2 - b1
    wm = np.zeros((KDIM, M), np.float32)
    for g in range(S):
        r0, c0 = g * AUG, g * D
        # out_j = sum_k x_k*W1[j,k] + sum_k (r*x)_k*Wd[j,k] + r*bd_j (+ b1_j)
        wm[r0:r0 + D, c0:c0 + D] = W1.T
        wm[r0 + D:r0 + 2 * D, c0:c0 + D] = Wd.T
        wm[r0 + 2 * D, c0:c0 + D] = bd
    bvec = np.tile(b1, S).astype(np.float32).reshape(M, 1)
    return wm.astype(NPBF16), bvec


def make_pe_inputs(x, route, tc_tokens, nt):
    """Host planarize: per-core [nt, KDIM, FTILE] bf16 aug arrays."""
    n_pad = nt * S * FTILE
    r = route.astype(np.float32)
    aug = np.empty((x.shape[0], AUG), NPBF16)
    aug[:, :D] = x
    aug[:, D:2 * D] = x * r[:, None]
    aug[:, 2 * D] = r
    per_core = []
    for c in range(N_CORES):
        a = aug[c * tc_tokens:(c + 1) * tc_tokens]
        if n_pad != tc_tokens:
            a = np.concatenate(
                [a, np.zeros((n_pad - tc_tokens, AUG), NPBF16)], axis=0)
        # [nt, S, FTILE, AUG] -> [nt, S, AUG, FTILE] -> [nt, KDIM, FTILE]
        a = np.ascontiguousarray(
            a.reshape(nt, S, FTILE, AUG).transpose(0, 1, 3, 2)
        ).reshape(nt, KDIM, FTILE)
        per_core.append(a)
    return per_core


def unplanarize(o, tc_tokens, nt):
    """[nt, M, FTILE] bf16 planar -> [tc_tokens, D] f32 token-major."""
    o = np.asarray(o).reshape(nt, S, D, FTILE).transpose(0, 1, 3, 2)
    return o.reshape(nt * S * FTILE, D)[:tc_tokens].astype(np.float32)


def kernel(x, W1, b1, W2, b2, route):
    x = np.asarray(x, np.float32)
    route = np.asarray(route)
    W1, b1 = np.asarray(W1, np.float32), np.asarray(b1, np.float32)
    W2, b2 = np.asarray(W2, np.float32), np.asarray(b2, np.float32)
    tc_tokens = x.shape[0] // N_CORES
    in_maps, perms = make_sorted2_inputs(x, route, tc_tokens, W1, b1, W2, b2)
    nc = build_moe_sorted2()
    res = run_bass_kernel_spmd(nc, in_maps, core_ids=list(range(N_CORES)))
    return np.concatenate(
        [unsort2_output(res.results[c], perms[c], tc_tokens)
         for c in range(N_CORES)], axis=0)


# ---------------------------------------------------------------------------
# Previous all-DVE implementation, kept for same-process A/B benchmarking.
# ---------------------------------------------------------------------------

def build_moe_v3(tc_tokens, W1, b1, W2, b2, r_tile=256, reps=1, gp_tiles=2,
                 layout="new"):
    """v3: DVE multiply-accumulate chain + Scalar-engine glue (old baseline)."""
    D = 10
    R = r_tile
    assert tc_tokens % (P * R) == 0
    nt = tc_tokens // (P * R)
    AF = mybir.ActivationFunctionType

    nc = bacc.Bacc("TRN2", target_bir_lowering=False, debug=False,
                   num_devices=N_CORES)
    x_ext = nc.dram_tensor("x", [tc_tokens, D], F32, kind="ExternalInput")
    r_ext = nc.dram_tensor("route", [tc_tokens], I32, kind="ExternalInput")
    w_ext = nc.dram_tensor("wt", [P, 220], F32, kind="ExternalInput")
    o_ext = nc.dram_tensor("out", [tc_tokens, D], F32, kind="ExternalOutput")

    xv = x_ext.rearrange("(n p r) d -> n p r d", p=P, r=R)
    rv = r_ext.rearrange("(n p r) -> n p r", p=P, r=R)
    ov = o_ext.rearrange("(n p r) d -> n p r d", p=P, r=R)

    with TileContext(nc) as tc:
        n_bufs = (4 if R <= 256 else 3) if layout == 'new' else 3
        with tc.tile_pool(name="const", bufs=1) as cpool, \
             tc.tile_pool(name="sbuf", bufs=n_bufs) as pool:
            wt = cpool.tile([P, 220], F32)
            nc.sync.dma_start(out=wt[:], in_=w_ext[:])

            def ap_wd(j, k):
                return wt[:, j * 10 + k:j * 10 + k + 1]

            def ap_w1(j, k):
                return wt[:, 100 + j * 10 + k:100 + j * 10 + k + 1]

            def ap_bd(j):
                return wt[:, 200 + j:200 + j + 1]

            def ap_b1(j):
                return wt[:, 210 + j:210 + j + 1]

            for _ in range(reps):
                for i in range(nt):
                    xt = pool.tile([P, R, D], F32, tag="xt")
                    rt = pool.tile([P, R], I32, tag="rtf")
                    nc.sync.dma_start(out=xt[:], in_=xv[i])
                    nc.sync.dma_start(out=rt[:], in_=rv[i])

                    rf = pool.tile([P, R], F32, tag="rtf")
                    nc.vector.tensor_copy(out=rf[:], in_=rt[:])

                    xp = pool.tile([P, D, R], F32, tag="xp")
                    for k in range(D):
                        nc.scalar.copy(out=xp[:, k, :], in_=xt[:, :, k])

                    accp = pool.tile([P, D, R], F32, tag="accp")
                    for j in range(D):
                        aj = accp[:, j, :]
                        nc.vector.tensor_scalar(
                            out=aj, in0=xp[:, 0, :], scalar1=ap_wd(j, 0),
                            scalar2=ap_bd(j), op0=ALU.mult, op1=ALU.add)
                        for k in range(1, D):
                            nc.vector.scalar_tensor_tensor(
                                out=aj, in0=xp[:, k, :], scalar=ap_wd(j, k),
                                in1=aj, op0=ALU.mult, op1=ALU.add)
                        nc.vector.tensor_mul(out=aj, in0=aj, in1=rf[:])
                        for k in range(D):
                            nc.vector.scalar_tensor_tensor(
                                out=aj, in0=xp[:, k, :], scalar=ap_w1(j, k),
                                in1=aj, op0=ALU.mult, op1=ALU.add)
                    acc = pool.tile([P, R, D], F32, tag="xt")
                    for j in range(D):
                        nc.scalar.activation(out=acc[:, :, j],
                                             in_=accp[:, j, :],
                                             func=AF.Identity, bias=ap_b1(j),
                                             scale=1.0)
                    nc.sync.dma_start(out=ov[i], in_=acc[:])
    nc.compile()
    return nc


def make_wt(W1, b1, W2, b2):
    Wd = (W2 - W1)
    bdv = (b2 - b1)
    cols = np.concatenate([Wd.reshape(-1), W1.reshape(-1), bdv, b1]
                          ).astype(np.float32)
    return np.tile(cols[None, :], (P, 1))
